# revision 1
# baseline (speedup 1.0000x reference)
"""Multi-head attention (B=4, S=2048, E=768, H=12) on 8 trn2 NeuronCores.

Sharding: tensor-parallel over heads x data-parallel over batch. Core c
handles batch b=c//2 and heads 6*(c%2)..6*(c%2)+5 (all 2048 queries). Each
core emits a partial output projection (its 6 heads' contribution); the two
cores of a batch pair are summed on the host during unsharding. The bias is
added on device by the even core only (odd cores receive a zero bias).

Layouts: matmul operands keep "feature on partitions" so that
  - the qkv projection emits Q^T/K^T directly (lhsT=w^T chunk, rhs=x^T chunk),
  - Q@K^T emits S^T = [k, q] (lhsT=K^T slice, rhs=Q^T slice, contract d=64),
  - softmax row sums come from a ones-column appended to V (AV matmul M=65),
  - attention output lands as outT [e', q] - exactly the lhsT the output
    projection wants.

Schedule: the kernel is jointly limited by the PE matmul stream and the
Scalar engine's exp (25.2M logits/core at ~0.83ns/elem), so the exp stream
must start early and never stall.
  - Prefix: x tiles stream in (3-deep prefetch), are cast fp32->fp16 on the
    otherwise-idle ScalarE, transposed on the PE at 1 cyc/row, and consumed
    immediately by the woven V projection plus only the Q/K chunks the
    first heads need (K f-tile 0, Q f-tile 0 half 0).
  - Stage B: one flat stream of 192 groups (halves-outer). Each group is
    one S^T matmul pair feeding one [128,1024] exp ACTIVATE. The AV
    accumulation runs LAG=3 groups behind the exp stream, so its wait on
    exp-complete is always pre-satisfied and the in-order PE queue never
    blocks the next S pair (this removes a cross-engine dependency cycle
    worth ~85ns/group). The 18 remaining Q/K chunks are woven into the
    stream at deadline-slack positions. Each head-half's accumulator is
    evacuated PSUM->SBUF right after its last AV so the single-buffer o
    rotation never waits on the normalization's DRAM bounce.
    PSUM: s 2x[128,1024] (4 banks) + o [128,1024] (2) + qk [128,512] (1).
  - Softmax normalization divides via a reciprocal reshaped
    [1,512]->[128,4] through a DRAM bounce (DVE reciprocal cost is
    free-size x lanes); the same bounce provides the partition broadcast
    of 1/sum. exp() folds the 1/sqrt(d) scale into the activation's affine
    input; max-subtraction is skipped (logits are ~N(0,1), exp cannot
    overflow).
  - Stage C: output projection tail + bias adds + store.

Dtypes: all matmul operands are fp16 (1 cyc/row like bf16 but 4x the
mantissa; every intermediate here is well inside fp16 range) with fp32
PSUM accumulation.

Environment workarounds (this walrus build): sync-waits are split one per
instruction onto NoOps (_split_waits, _TC).
"""

import numpy as np

import concourse.bass as bass
import concourse.tile as tile
from concourse import mybir
from concourse.bass_utils import run_bass_kernel_spmd
from concourse.masks import make_identity
from concourse.tile import ScopedClock

B, S, E, H, D = 4, 2048, 768, 12, 64
NCORES = 8
HL = 6               # heads per core
FL = HL * D          # 384 local feature dim
SCALE = D ** -0.5
FP = mybir.dt.float32
FR = mybir.dt.float32r
F16 = mybir.dt.float16
BF = mybir.dt.bfloat16
P = 128

ET = E // P          # 6 e-chunks of 128
FT = FL // P         # 3 local f-tiles of 128
NKT = S // P         # 16 k-tiles of 128
NQC = S // 512       # 4 q-chunks of 512
NST = S // P         # 16 s-tiles
DV = D + 1           # 65: V plus ones column


class _TC(tile.TileContext):
    """TileContext with the end-of-kernel drain's sem waits split one per
    instruction (this walrus build's CTRL_NO_STRUCT encoding holds only one
    sync wait; the stock drain carries one wait per outstanding proc)."""

    def _drain_and_barrier(self, tick_clock, wait_clock):
        probe = self.nc.sync.nop()
        wait_clock.add_sem_waits(
            probe.ins, ScopedClock({None: tick_clock.global_clock})
        )
        si = probe.ins.sync_info
        waits = list(si.on_wait) if si is not None else []
        if len(waits) > 1:
            si.on_wait = waits[:1]
            for w in waits[1:]:
                n = self.nc.sync.nop()
                n.ins.sync_info = type(si)(on_wait=[w], on_update=[])
        self.nc.sync.drain()
        self.nc.all_engine_barrier()
        popped = self.nc._tile_sem_poison_stack.pop()
        assert popped is self._sem_poison
        self.nc.clear_and_free_semaphores(list(self.sems.allocated().values()))
        self.nc.all_engine_barrier()


def _split_waits(nc):
    """This walrus build accepts at most one sync-wait per TPB instruction
    (two on EventSemaphore). Tile emits up to 2-3. Hoist the extras onto
    same-engine NoOps inserted immediately before the instruction."""
    ctr = [0]
    for f in nc.m.functions:
        for bb in f.blocks:
            out = []
            changed = False
            for inst in bb.instructions:
                si = getattr(inst, "sync_info", None)
                if si is not None and si.on_wait:
                    cap = 2 if isinstance(inst, mybir.InstEventSemaphore) else 1
                    waits = list(si.on_wait)
                    if len(waits) > cap:
                        changed = True
                        for w in waits[:-cap]:
                            ctr[0] += 1
                            out.append(
                                mybir.InstNoOp(
                                    name=f"WSPLIT-{ctr[0]}",
                                    engine=inst.engine,
                                    ins=[],
                                    outs=[],
                                    sync_info=mybir.SyncInfo(
                                        on_wait=[w], on_update=[]
                                    ),
                                    bass_nofuse=True,
                                )
                            )
                        si.on_wait = waits[-cap:]
                        inst.sync_info = si
                out.append(inst)
            if changed:
                bb.instructions = out


def build(n_reps=1):
    nc = bass.Bass()
    xb = nc.dram_tensor("xb", [S, E], FP, kind="ExternalInput")
    wqkvT = nc.dram_tensor("wqkvT", [E, 3 * FL], F16, kind="ExternalInput")
    wprojT = nc.dram_tensor("wprojT", [FL, E], F16, kind="ExternalInput")
    biasb = nc.dram_tensor("biasb", [P, E], FP, kind="ExternalInput")
    identd = nc.dram_tensor("identd", [P, P], F16, kind="ExternalInput")
    out = nc.dram_tensor("out", [S, E], FP, kind="ExternalOutput")

    Exp = mybir.ActivationFunctionType.Exp

    from contextlib import ExitStack

    with _TC(nc) as tc, ExitStack() as stack:
        consts = stack.enter_context(tc.tile_pool(name="consts", bufs=1))
        persist = stack.enter_context(tc.tile_pool(name="persist", bufs=1))

        ident = consts.tile([P, P], F16)
        nc.sync.dma_start(ident[:], identd[:])
        bias_sb = consts.tile([P, E], FP)

        wproj_sb = [
            consts.tile([P, E], F16, tag=f"wproj{c}", name=f"wproj{c}")
            for c in range(FT)
        ]


        wqkv_sb = [
            consts.tile([P, 3 * FL], F16, tag=f"wqkv{c}", name=f"wqkv{c}")
            for c in range(ET)
        ]
        xbT = [
            persist.tile([P, S], F16, tag=f"xbT{c}", name=f"xbT{c}")
            for c in range(ET)
        ]

        # persistent activations
        qT = [persist.tile([P, S], F16, tag=f"qT{t}", name=f"qT{t}") for t in range(FT)]
        kT = [persist.tile([P, S], F16, tag=f"kT{t}", name=f"kT{t}") for t in range(FT)]
        vp = [persist.tile([P, HL * DV], F16, tag=f"vp{t}", name=f"vp{t}") for t in range(NST)]
        outT = [persist.tile([P, S], F16, tag=f"outT{t}", name=f"outT{t}") for t in range(FT)]

        for _rep in range(n_reps):
            # ---------------- Stage A: transposes + projections ----------------
            with tc.tile_pool(name="stagea", bufs=1) as stagea, \
                 tc.tile_pool(name="xload", bufs=4) as xload, \
                 tc.tile_pool(name="tr_psum", bufs=4, space="PSUM") as tr_psum, \
                 tc.tile_pool(name="mm_psum", bufs=3, space="PSUM") as mm_psum:

                def qk_chunk(which, ft, j):
                    # one 512-column chunk of Q^T (which=0) or K^T (which=1)
                    # for f-tile ft; needs x tiles 4j..4j+3 transposed
                    dst = qT if which == 0 else kT
                    pq = mm_psum.tile([P, 512], FP, tag="mm", name=f"pq{which}_{ft}_{j}")
                    for c in range(ET):
                        nc.tensor.matmul(
                            pq[:],
                            (wqkv_sb[c][:, FL * which + P * ft : FL * which + P * (ft + 1)]),
                            (xbT[c][:, 512 * j : 512 * (j + 1)]),
                            start=(c == 0),
                            stop=(c == ET - 1),
                        )
                    nc.vector.tensor_copy(dst[ft][:, 512 * j : 512 * (j + 1)], pq[:])

                def v_tile(t):
                    pv = mm_psum.tile([P, 512], FP, tag="mm", name=f"pv{t}")
                    for c in range(ET):
                        nc.tensor.matmul(
                            pv[:, :FL],
                            (xbT[c][:, P * t : P * (t + 1)]),
                            (wqkv_sb[c][:, 2 * FL : 3 * FL]),
                            start=(c == 0),
                            stop=(c == ET - 1),
                        )
                    v3 = vp[t].rearrange("p (h d) -> p h d", d=DV)
                    nc.vector.tensor_copy(
                        v3[:, :, 0:D], pv[:, :FL].rearrange("p (h d) -> p h d", d=D)
                    )
                    nc.vector.memset(v3[:, :, D : D + 1], 1.0)

                # Prefix: x^T via fp16 PE transpose (casts on ScalarE), V per
                # tile as it lands, and only the Q/K chunks the first heads
                # need (K f-tile 0 fully, Q f-tile 0 for the first q-half).
                # All remaining Q/K chunks are woven into stage B's PE idle.
                Copy = mybir.ActivationFunctionType.Copy
                xts = []
                for t in range(5):
                    xt = xload.tile([P, E], FP, tag="xt", bufs=6, name=f"xt{t}")
                    nc.sync.dma_start(xt[:], xb[P * t : P * (t + 1), :])
                    xts.append(xt)
                for t in range(NST):
                    x16 = xload.tile([P, E], F16, tag="x16", bufs=3, name=f"x16_{t}")
                    nc.scalar.activation(x16[:], xts[t][:], Copy)
                    if t == 0:
                        # V's weight columns first: the V tiles woven into
                        # the transpose loop consume them almost immediately
                        for c in range(ET):
                            nc.scalar.dma_start(
                                wqkv_sb[c][:, 2 * FL : 3 * FL],
                                wqkvT[P * c : P * (c + 1), 2 * FL : 3 * FL],
                            )
                    elif t == 1:
                        # K then Q columns: first consumers are the K/Q
                        # chunks at t=3
                        for c in range(ET):
                            nc.scalar.dma_start(
                                wqkv_sb[c][:, FL : 2 * FL],
                                wqkvT[P * c : P * (c + 1), FL : 2 * FL],
                            )
                    elif t == 2:
                        for c in range(ET):
                            nc.scalar.dma_start(
                                wqkv_sb[c][:, 0:FL],
                                wqkvT[P * c : P * (c + 1), 0:FL],
                            )
                    elif t == 3:
                        for c in range(FT):
                            nc.scalar.dma_start(
                                wproj_sb[c][:], wprojT[P * c : P * (c + 1), :]
                            )
                        nc.scalar.dma_start(bias_sb[:], biasb[:])
                    for c in range(ET):
                        pt = tr_psum.tile([P, P], F16, tag="tr")
                        nc.tensor.transpose(pt[:], x16[:, P * c : P * (c + 1)], ident[:])
                        nc.vector.tensor_copy(xbT[c][:, P * t : P * (t + 1)], pt[:])
                    if t + 5 < NST:
                        xt = xload.tile([P, E], FP, tag="xt", bufs=6, name=f"xt{t+5}")
                        nc.sync.dma_start(xt[:], xb[P * (t + 5) : P * (t + 6), :])
                        xts.append(xt)
                    v_tile(t)
                    if t % 4 == 3:
                        qk_chunk(1, 0, t // 4)
                        if t // 4 < 2:
                            qk_chunk(0, 0, t // 4)

            # ---------------- Stage B: attention, half-sequence strips ------
            # One flat stream of 192 groups (halves-outer). S^T feeds the exp
            # stream; AV runs LAG groups behind so its exp-complete wait is
            # always pre-satisfied and the in-order PE never stalls. The
            # remaining Q/K projection chunks are woven into the stream at
            # deadline-slack positions, using a dedicated 1-bank psum tag.
            # PSUM: s 2x2 + o 3 + qk 1 = 8 banks. Each head-half's
            # accumulators are evacuated to SBUF right after their last AV
            # so the 3-slot o rotation never waits on the slow
            # normalization DMA bounce.
            HQ = S // 2  # 1024 queries per strip
            with tc.tile_pool(name="s_psum", bufs=2, space="PSUM") as s_psum, \
                 tc.tile_pool(name="o_psum", bufs=3, space="PSUM") as o_psum, \
                 tc.tile_pool(name="qk_psum", bufs=1, space="PSUM") as qk_psum, \
                 tc.tile_pool(name="expst", bufs=8) as expst, \
                 tc.tile_pool(name="smalls", bufs=6) as smalls, \
                 tc.tile_pool(name="invdram", bufs=6, space="DRAM") as invdram:

                def b_chunk(which, ft, j):
                    dst = qT if which == 0 else kT
                    pq = qk_psum.tile([P, 512], FP, tag="qk", name=f"bq{which}_{ft}_{j}")
                    for c in range(ET):
                        nc.tensor.matmul(
                            pq[:],
                            (wqkv_sb[c][:, FL * which + P * ft : FL * which + P * (ft + 1)]),
                            (xbT[c][:, 512 * j : 512 * (j + 1)]),
                            start=(c == 0),
                            stop=(c == ET - 1),
                        )
                    nc.vector.tensor_copy(dst[ft][:, 512 * j : 512 * (j + 1)], pq[:])

                # chunk -> weave position; deadlines: K ft1 by gi 32, K ft2 &
                # Q ft2 j0/j1 by 64, Q ft0 j2/j3 by 96 (h0 half1), Q ft1
                # j2/j3 by 128, Q ft2 j2/j3 by 160.
                weave = {
                    4: (1, 1, 0), 9: (1, 1, 1), 14: (1, 1, 2), 19: (1, 1, 3),
                    24: (0, 1, 0), 28: (0, 1, 1),
                    36: (1, 2, 0), 41: (1, 2, 1), 46: (1, 2, 2), 51: (1, 2, 3),
                    56: (0, 2, 0), 60: (0, 2, 1),
                    66: (0, 0, 2), 70: (0, 0, 3),
                    100: (0, 1, 2), 104: (0, 1, 3),
                    132: (0, 2, 2), 136: (0, 2, 3),
                }

                def norm(h, half, posb, mul_eng=None):
                    ht, hb = (D * h) // P, (D * h) % P
                    mul_eng = mul_eng or nc.vector
                    q0 = HQ * half
                    for j in range(2):
                        # sums row -> DRAM, reread as [128,4] so the
                        # reciprocal runs 128-wide (free-size cost), then
                        # bounce back through DRAM for the partition
                        # broadcast (SBUF sources can't have step-0
                        # partitions; DRAM can).
                        sd = invdram.tile([1, 512], FP, tag="sd")
                        nc.sync.dma_start(sd[:], posb[j][D : D + 1, :])
                        s4 = smalls.tile([P, 4], FP, tag="s4")
                        nc.sync.dma_start(
                            s4[:], sd.rearrange("a (p f) -> (a p) f", p=P)
                        )
                        inv4 = smalls.tile([P, 4], FP, tag="inv4")
                        nc.vector.reciprocal(inv4[:], s4[:])
                        invd = invdram.tile([1, 512], FP, tag="invd")
                        nc.sync.dma_start(
                            invd.rearrange("a (p f) -> (a p) f", p=P), inv4[:]
                        )
                        inv64 = smalls.tile([D, 512], FP, tag="inv64")
                        nc.sync.dma_start(
                            inv64[:], invd[0:1, :].to_broadcast((D, 512))
                        )
                        mul_eng.tensor_mul(
                            outT[ht][hb : hb + D, q0 + 512 * j : q0 + 512 * (j + 1)],
                            posb[j][0:D, :],
                            inv64[:],
                        )

                def emit_av(h, half, i, es):
                    if i == 0:
                        po_of[(h, half)] = [
                            o_psum.tile([P, 512], FP, tag="o", name=f"po{h}_{half}_{j}")
                            for j in range(2)
                        ]
                    po = po_of[(h, half)]
                    if i < NKT - 1:
                        for j in range(2):
                            nc.tensor.matmul(
                                po[j][:DV, :],
                                vp[i][:, DV * h : DV * (h + 1)],
                                es[:, 512 * j : 512 * (j + 1)],
                                start=(i == 0),
                                stop=False,
                            )
                    else:
                        posb = []
                        for j in range(2):
                            nc.tensor.matmul(
                                po[j][:DV, :],
                                vp[i][:, DV * h : DV * (h + 1)],
                                es[:, 512 * j : 512 * (j + 1)],
                                start=False,
                                stop=True,
                            )
                            pc = smalls.tile(
                                [DV, 512], FP, tag="posb", bufs=4,
                                name=f"posb{h}_{half}_{j}",
                            )
                            nc.vector.tensor_copy(pc[:], po[j][:DV, :])
                            posb.append(pc)
                        del po_of[(h, half)]
                        last = (h, half) == groups[-1][:2]
                        norm(h, half, posb, mul_eng=nc.gpsimd if last else None)

                LAG = 3
                groups = [
                    (h, half, i)
                    for half in range(2)
                    for h in range(HL)
                    for i in range(NKT)
                ]
                po_of, esq = {}, []
                for gi, (h, half, i) in enumerate(groups):
                    ht, hb = (D * h) // P, (D * h) % P
                    q0 = HQ * half
                    ps = s_psum.tile([P, HQ], FP, tag="s")
                    for j in range(2):
                        nc.tensor.matmul(
                            ps[:, 512 * j : 512 * (j + 1)],
                            (kT[ht][hb : hb + D, P * i : P * (i + 1)]),
                            (qT[ht][hb : hb + D, q0 + 512 * j : q0 + 512 * (j + 1)]),
                            start=True,
                            stop=True,
                        )
                    es = expst.tile([P, HQ], F16, tag="e")
                    nc.scalar.activation(es[:], ps[:], Exp, scale=SCALE)
                    esq.append(es)
                    if gi in weave:
                        b_chunk(*weave[gi])
                    if gi >= LAG:
                        emit_av(*groups[gi - LAG], esq[gi - LAG])
                for gi in range(len(groups) - LAG, len(groups)):
                    emit_av(*groups[gi], esq[gi])

            # ---------------- Stage C: partial output projection + bias --------
            with tc.tile_pool(name="f_psum", bufs=2, space="PSUM") as f_psum, \
                 tc.tile_pool(name="osb", bufs=4) as osb:
                for t in range(NST):
                    pf1 = f_psum.tile([P, 512], FP, tag="f1")
                    pf2 = f_psum.tile([P, 512], FP, tag="f2")
                    for c in range(FT):
                        nc.tensor.matmul(
                            pf1[:],
                            (outT[c][:, P * t : P * (t + 1)]),
                            (wproj_sb[c][:, 0:512]),
                            start=(c == 0),
                            stop=(c == FT - 1),
                        )
                    for c in range(FT):
                        nc.tensor.matmul(
                            pf2[:, :256],
                            (outT[c][:, P * t : P * (t + 1)]),
                            (wproj_sb[c][:, 512:E]),
                            start=(c == 0),
                            stop=(c == FT - 1),
                        )
                    ot = osb.tile([P, E], FP, tag="ot")
                    nc.vector.tensor_add(ot[:, 0:512], pf1[:], bias_sb[:, 0:512])
                    nc.vector.tensor_add(ot[:, 512:E], pf2[:, :256], bias_sb[:, 512:E])
                    nc.sync.dma_start(out[P * t : P * (t + 1), :], ot[:])

    _split_waits(nc)
    return nc


_CACHE = {}


def _get_nc():
    if "nc" not in _CACHE:
        _CACHE["nc"] = build()
    return _CACHE["nc"]


def make_in_maps(x, w_qkv, w_proj, b_proj):
    x = np.asarray(x, dtype=np.float32)
    w_qkv = np.asarray(w_qkv, np.float32)
    w_proj = np.asarray(w_proj, np.float32)
    b_proj = np.asarray(b_proj, np.float32)
    bias0 = np.ascontiguousarray(np.broadcast_to(b_proj, (P, E)))
    biasz = np.zeros((P, E), np.float32)
    in_maps = []
    for c in range(NCORES):
        b, half = c // 2, c % 2
        heads = range(HL * half, HL * half + HL)
        rows = (
            [E * 0 + D * h + d for h in heads for d in range(D)]
            + [E * 1 + D * h + d for h in heads for d in range(D)]
            + [E * 2 + D * h + d for h in heads for d in range(D)]
        )
        wqkvT_l = np.ascontiguousarray(w_qkv[rows, :].T).astype(np.float16)
        wprojT_l = np.ascontiguousarray(w_proj[:, rows[: FL]].T).astype(np.float16)
        in_maps.append(
            {
                "identd": np.eye(P, dtype=np.float16),
                "xb": np.ascontiguousarray(x[b]),
                "wqkvT": wqkvT_l,
                "wprojT": wprojT_l,
                "biasb": bias0 if half == 0 else biasz,
            }
        )
    return in_maps


def assemble(results):
    outp = np.empty((B, S, E), np.float32)
    for b in range(B):
        outp[b] = results[2 * b]["out"] + results[2 * b + 1]["out"]
    return outp


def kernel(x, w_qkv, w_proj, b_proj):
    nc = _get_nc()
    in_maps = make_in_maps(x, w_qkv, w_proj, b_proj)
    res = run_bass_kernel_spmd(nc, in_maps, core_ids=list(range(NCORES)))
    return assemble(res.results)

